# revision 12
# baseline (speedup 1.0000x reference)
"""Trainium2 Bass kernel for nn_ModelR_37022618091886.

Model: y = MLP(x) with 5 layers (leaky-relu 0.01 between), then per-example
triangular scatter of the 2080 outputs into an upper-triangular 64x64 matrix
(diagonal entries abs'ed), output shape (64, 64, 8192).

Strategy:
  - Data-parallel over batch across 8 cores (1024 examples/core), weights
    replicated.
  - Activations are kept feature-major on chip: h^T [features, batch], so the
    contraction dim of every matmul is already on SBUF partitions and the
    final layer directly produces y^T [2080, batch] = (almost) the output.
  - Matmuls run in bf16 (full PE rate + FWL weight loads; measured rel err
    ~5e-3 vs the f32 reference, well inside the 2e-2 gate).  PSUM/bias/output
    stay f32.  Weights are pre-tiled on the host to [MT*128, KT*128] so each
    m-tile's lhsT k-tiles load as one fully-contiguous [128, KT*128] DMA.
  - W3's columns are permuted on the host so that the final layer's output
    rows land in output-row order: row block i holds (i,j) for j=i..63
    ascending.  The "scatter" then degenerates to ~80 contiguous-run DMAs,
    spread round-robin over the Pool (SWDGE) queue and both HWDGE rings so
    the L5 weight stream is never blocked.  L5 m-tiles run in reverse order
    so the many-small-run tiles drain under later compute.
  - The diagonal abs is y = max(y, c*y) with c = -1 on diagonal rows else +1
    (per-partition scalar), fused after the bias add.
  - Strict-lower-triangle rows of the output are never written; the runtime
    pre-zeroes/donates zeroed output buffers (see run_bass_via_pjrt), so they
    read back as exact zeros.
  - PE is ~98% busy in the CoreSim timing model (499us of matmul stream per
    rep); the remaining ~12us are the x/w1 startup fill and the last
    m-tile's ACT->DVE->store tail.

HW findings (measured via microbenchmarks, 2026-08):
  - With all 8 cores running full PE streams the chip power-throttles the
    PE clock to ~2.0 GHz (per-MM N=512 ~250-265ns vs the 213.3ns 2.4 GHz
    ideal; a single busy core measures ~217ns/MM).  2336 matmuls x ~255ns
    = ~596us is the de-facto floor; the kernel sits on it.
  - LDWEIGHTS is fully hidden by the PE's pull-ahead even at one load per
    matmul (a kernel with a single LDW total measures the same per-MM).
    The legalizer still emits a redundant LDW for the second half-matmul
    of each (m,k) pair; _dedup_ldweights removes those 1168 (harmless now,
    helps if the clock ever runs unthrottled).
  - PSUM bank alternation per k (bank0/bank1 within a pair) has no HW
    penalty: sequential same-bank chains measure the same (seq2 variant).
  - fp8-e4m3 DoubleRow (1.44x PE rate) fails the 2e-2 gate: measured
    rel err 6-8e-2 end-to-end (even fp8 on only the middle layers), so
    bf16 dense is the fastest admissible algorithm.
  - Core-count sweep (6/7/8 busy cores) shows no per-MM difference, so
    8-way batch sharding (least work per core) is optimal; the clock
    state drifts ~20% on hour timescales regardless.
  - For reps > 1 (timing builds), the next rep's x tiles + w1-m0 are
    prefetched before the L5 store traffic so the next L1 doesn't stall
    behind ~8.5MB of output stores in the DMA ring FIFOs.  The w1
    prefetch uses its own pool tag ("w1pf"): sharing tag "w" would let
    L5's weight rotation claim the buffer and deadlock the PE queue.
"""

import os

import numpy as np

CPV = 64
L = CPV * (CPV + 1) // 2  # 2080
LT = 17  # number of 128-row tiles covering L (padded)
LP = LT * 128  # 2176
D_IN = 1024
H = 2048
B = 8192
N_CORES = 8
BC = B // N_CORES  # 1024 batch per core
NCH = 2  # moving-dim chunks of 512 (fp32 PSUM bank limit)
NSZ = BC // NCH

_DTYPE = os.environ.get("KERNEL_DTYPE", "bf16")  # "bf16" | "f32r" | "f32"
_REPS = int(os.environ.get("KERNEL_REPS", "1"))  # timing aid: unroll kernel R times
# matmul emission order within an m-tile:
#   kmajor   : per k, two N=512 matmuls alternating PSUM banks (original)
#   halfmajor: two sequential same-bank chains of kt matmuls (measured on HW:
#              no faster — there is no per-matmul PSUM bank-switch penalty)
_MM_ORDER = os.environ.get("KERNEL_MM_ORDER", "kmajor")
# N=1024 bf16 moving operands were tried (would halve the matmul/Ldweights
# instruction count) but neuronxcc rejects the 2-bank f32 PSUM output with an
# ISA check failure on TRN2 — the 128x1024 moving max evidently requires
# TRN3's 16-bit PSUM accumulation. Keep off.
_N1024 = _DTYPE == "bf16" and os.environ.get("KERNEL_N1024", "0") == "1"

_compiled_nc = None

# Remove the redundant second InstLdweights the legalizer emits for the
# b-half matmul of each (m,k) pair (identical weights AP, only matmuls in
# between on the PE queue).  Saves ~30ns of unhidden weight-load time per
# pair on HW.
_LDW_DEDUP = os.environ.get("KERNEL_LDW_DEDUP", "1") == "1"


def _dedup_ldweights(ordered):
    import concourse.mybir as mybir

    removed = 0
    rename = {}
    for bb, insts in ordered.items():
        keep = []
        last_sig = None
        last_name = None
        for inst in insts:
            if isinstance(inst, mybir.InstLdweights):
                sig = (
                    str(inst.ins[0]),
                    str(inst.perf_mode),
                    str(inst.is_transpose),
                    str(inst.tile_position),
                )
                if sig == last_sig and last_name is not None:
                    rename[inst.name] = last_name
                    removed += 1
                    continue
                last_sig = sig
                last_name = inst.name
            elif getattr(inst, "engine", None) == mybir.EngineType.PE:
                if not isinstance(inst, mybir.InstMatmult):
                    last_sig = None
                    last_name = None
            keep.append(inst)
        ordered[bb] = keep
    if rename:
        for bb, insts in ordered.items():
            for inst in insts:
                deps = set(inst.sync_dependency_names()) | set(
                    inst.nosync_dependency_names()
                )
                hit = {d: rename[d] for d in deps if d in rename}
                if hit:
                    inst.remap_dependency_names(hit)
    return ordered


def _install_dedup_patch(enable):
    import concourse.tile as tile_mod

    if not getattr(tile_mod, "_ldw_dedup_installed", False):
        orig = tile_mod.tile_legalize

        def patched(ordered, nc):
            out = orig(ordered, nc)
            if getattr(tile_mod, "_ldw_dedup_enabled", False):
                out = _dedup_ldweights(out)
            return out

        tile_mod.tile_legalize = patched
        tile_mod._ldw_dedup_installed = True
    tile_mod._ldw_dedup_enabled = enable


def _offsets():
    off = np.zeros(CPV + 1, dtype=np.int64)
    for i in range(CPV):
        off[i + 1] = off[i] + (CPV - i)
    return off


def _perm_and_coefs():
    """Column permutation for W3 + diag coefficient vector.

    New output order m: for i in 0..63, for j in i..63 -> m = off[i] + (j-i).
    Torch/ref order k: for i, for idx_y: col j = 63-idx_y -> k = off[i] + (63-j).
    """
    off = _offsets()
    perm = np.empty(L, dtype=np.int64)
    for i in range(CPV):
        n = CPV - i
        perm[off[i] : off[i] + n] = off[i] + np.arange(n)[::-1]
    coef = np.ones(LP, dtype=np.float32)
    coef[off[:CPV]] = -1.0  # diagonal (i,i) sits at the start of block i
    return perm, coef


def _out_runs():
    """Maximal runs of consecutive final-layer rows that map to consecutive
    output rows: list of (tile, p0, length, out_row0)."""
    off = _offsets()
    runs = []
    for t in range(LT):
        m0t, m1t = t * 128, min((t + 1) * 128, L)
        m = m0t
        while m < m1t:
            i = int(np.searchsorted(off, m, side="right") - 1)
            end = int(min(m1t, off[i] + (CPV - i)))
            runs.append((t, m - m0t, end - m, 65 * i + (m - off[i])))
            m = end
    assert sum(r[2] for r in runs) == L
    return runs


def _build(wp_bufs=6, ps_bufs=None, yp_bufs=3, reps=None, ldw_dedup=None):
    if ps_bufs is None:
        ps_bufs = 4  # [128,1024] f32 tiles span 2 banks each
    import concourse.bacc as bacc
    import concourse.mybir as mybir
    import concourse.tile as tile

    _install_dedup_patch(_LDW_DEDUP if ldw_dedup is None else ldw_dedup)

    F32 = mybir.dt.float32
    MMDT = {
        "bf16": mybir.dt.bfloat16,
        "f32r": mybir.dt.float32r,
        "f32": mybir.dt.float32,
    }[_DTYPE]
    ACT = mybir.ActivationFunctionType

    nc = bacc.Bacc("TRN2", target_bir_lowering=False, debug=False, num_devices=N_CORES)

    # weights are host-pre-tiled to [MT*128, KT*128]: row (m*128+p), col
    # (k*128+c) holds W[k*128+p, m*128+c], so loading one m-tile's lhsT
    # k-tiles is a single fully-contiguous [128, KT*128] DMA.
    xt = nc.dram_tensor("xt", [D_IN, BC], MMDT, kind="ExternalInput")
    w1 = nc.dram_tensor("w1", [H, D_IN], MMDT, kind="ExternalInput")
    w2 = nc.dram_tensor("w2", [H, H], MMDT, kind="ExternalInput")
    w21 = nc.dram_tensor("w21", [H, H], MMDT, kind="ExternalInput")
    w22 = nc.dram_tensor("w22", [H, H], MMDT, kind="ExternalInput")
    w3 = nc.dram_tensor("w3", [LP, H], MMDT, kind="ExternalInput")
    b1t = nc.dram_tensor("b1t", [128, H // 128], F32, kind="ExternalInput")
    b2t = nc.dram_tensor("b2t", [128, H // 128], F32, kind="ExternalInput")
    b21t = nc.dram_tensor("b21t", [128, H // 128], F32, kind="ExternalInput")
    b22t = nc.dram_tensor("b22t", [128, H // 128], F32, kind="ExternalInput")
    b3t = nc.dram_tensor("b3t", [128, LT], F32, kind="ExternalInput")
    c3t = nc.dram_tensor("c3t", [128, LT], F32, kind="ExternalInput")
    out = nc.dram_tensor("out", [CPV * CPV, BC], F32, kind="ExternalOutput")

    runs = _out_runs()

    with tile.TileContext(nc) as tc:
        with (
            tc.tile_pool(name="acts", bufs=1) as acts,
            tc.tile_pool(name="wp", bufs=wp_bufs) as wp,
            tc.tile_pool(name="cst", bufs=1) as cst,
            tc.tile_pool(name="yp", bufs=yp_bufs) as yp,
            tc.tile_pool(name="ycp", bufs=1) as ycp,
            tc.tile_pool(name="ps", bufs=ps_bufs, space="PSUM") as ps,
        ):
            # --- constants (biases / diag coefs): allocate now, load after
            # the startup-critical w1/x DMAs so they don't delay the first
            # Ldweights/matmul on any ring ---
            bias_specs = (
                ("b1", b1t, 16),
                ("b2", b2t, 16),
                ("b21", b21t, 16),
                ("b22", b22t, 16),
                ("b3", b3t, LT),
                ("c3", c3t, LT),
            )
            bias_tiles = {}
            for name, _dram, nt in bias_specs:
                t = cst.tile([128, nt], F32, tag=name, name=name)
                bias_tiles[name] = t

            n_reps = _REPS if reps is None else reps
            prefetched = None
            for _rep in range(n_reps):
              def load_weight_block(wt, w_dram, kt, m, lname):
                  """Load m-tile m's lhsT k-tiles: one contiguous [128, kt*128]
                  DMA, alternating between the two HWDGE rings per m-tile
                  (single DMA halves the completion-semaphore traffic)."""
                  eng = nc.sync if m % 2 == 0 else nc.scalar
                  eng.dma_start(wt[:], w_dram[m * 128 : (m + 1) * 128, : kt * 128])

              # startup: w1 m-tile 0 on the sync ring while x k0 loads in
              # parallel on the Pool ring; remaining x tiles (one DMA each)
              # round-robin over the three queues.  For reps > 1 these were
              # prefetched before the previous rep's L5 stores so they don't
              # queue behind them (w1-m0 gets its own pool tag "w1pf" so the
              # L5 weight rotation can't claim its buffer -> deadlock).
              x_engs = [nc.scalar, nc.gpsimd, nc.sync]
              if prefetched is None:
                  x_tiles = []
                  for k in range(D_IN // 128):
                    t = acts.tile([128, BC], MMDT, tag=f"x{k}", name=f"x{k}")
                    x_tiles.append(t)

                  w_l1_m0 = wp.tile([128, D_IN], MMDT, tag="w1pf", name="w_l1_0")
                  nc.sync.dma_start(w_l1_m0[:], w1[0:128, :D_IN])
                  nc.gpsimd.dma_start(x_tiles[0][:], xt[0:128, :])

                  for k in range(1, D_IN // 128):
                    x_engs[k % 3].dma_start(
                        x_tiles[k][:], xt[k * 128 : (k + 1) * 128, :]
                    )
              else:
                  x_tiles, w_l1_m0 = prefetched

              if _rep == 0:
                  # bias loads land behind the startup-critical DMAs, spread
                  # over the rings; b1 (first needed, at L1 m0's ACT ~6us in)
                  # leads on the sync ring
                  bias_engs = [nc.sync, nc.scalar, nc.gpsimd]
                  for bi, (name, dram, _nt) in enumerate(bias_specs):
                      bias_engs[bi % 3].dma_start(bias_tiles[name][:], dram[:, :])

              def layer(lname, w_dram, kt, mt, h_in, btile, out_tag_prefix, w_m0=None):
                  """One hidden layer: h_out[m] = lrelu(W[:,m]^T @ h_in + b[m])."""
                  h_out = []
                  for m in range(mt):
                      if m == 0 and w_m0 is not None:
                          wt = w_m0
                      else:
                          wt = wp.tile(
                              [128, kt * 128], MMDT, tag="w", name=f"w_{lname}_{m}"
                          )
                          load_weight_block(wt, w_dram, kt, m, lname)
                      if _N1024:
                          pst = ps.tile([128, BC], F32, tag="ps", name=f"ps_{lname}_{m}")
                          halves = [(pst[:, :NSZ], slice(0, NSZ)), (pst[:, NSZ:], slice(NSZ, BC))]
                          for k in range(kt):
                              nc.tensor.matmul(
                                  pst[:],
                                  wt[:, k * 128 : (k + 1) * 128],
                                  h_in[k][:, :],
                                  start=(k == 0),
                                  stop=(k == kt - 1),
                              )
                      else:
                          # one 2-bank PSUM tile per m-tile: each matmul's
                          # output stays within a single bank, but the two
                          # chains are adjacent so ONE activation reads both
                          pst = ps.tile([128, BC], F32, tag="ps", name=f"ps_{lname}_{m}")
                          halves = [(pst[:], slice(0, BC))]
                          if _MM_ORDER == "halfmajor":
                              # sequential same-bank chains: no per-matmul
                              # PSUM bank switch (measurably faster on HW)
                              for csl in (slice(0, NSZ), slice(NSZ, BC)):
                                  for k in range(kt):
                                      nc.tensor.matmul(
                                          pst[:, csl],
                                          wt[:, k * 128 : (k + 1) * 128],
                                          h_in[k][:, csl],
                                          start=(k == 0),
                                          stop=(k == kt - 1),
                                      )
                          else:
                              for k in range(kt):
                                  lhsT = wt[:, k * 128 : (k + 1) * 128]
                                  nc.tensor.matmul(
                                      pst[:, :NSZ],
                                      lhsT,
                                      h_in[k][:, :NSZ],
                                      start=(k == 0),
                                      stop=(k == kt - 1),
                                  )
                                  nc.tensor.matmul(
                                      pst[:, NSZ:],
                                      lhsT,
                                      h_in[k][:, NSZ:],
                                      start=(k == 0),
                                      stop=(k == kt - 1),
                                  )
                      ht = acts.tile(
                          [128, BC],
                          MMDT,
                          tag=f"{out_tag_prefix}{m}",
                          name=f"h_{lname}_{m}",
                      )
                      for ps_ap, csl in halves:
                          nc.scalar.activation(
                              ht[:, csl],
                              ps_ap,
                              ACT.Lrelu,
                              bias=btile[:, m : m + 1],
                              scale=1.0,
                              alpha=0.01,
                          )
                      h_out.append(ht)
                  return h_out

              h1 = layer(
                  "l1", w1, D_IN // 128, H // 128, x_tiles, bias_tiles["b1"], "a",
                  w_m0=w_l1_m0,
              )
              h2 = layer("l2", w2, H // 128, H // 128, h1, bias_tiles["b2"], "b")
              h3 = layer("l3", w21, H // 128, H // 128, h2, bias_tiles["b21"], "a")
              h4 = layer("l4", w22, H // 128, H // 128, h3, bias_tiles["b22"], "b")

              # --- prefetch next rep's x + w1-m0 ahead of the L5 store
              # traffic (they would otherwise sit behind ~8.5MB of output
              # stores in the DMA ring FIFOs and stall the next rep's L1) ---
              prefetched = None
              if _rep + 1 < n_reps:
                  nx = []
                  for k in range(D_IN // 128):
                      t = acts.tile([128, BC], MMDT, tag=f"x{k}", name=f"x{k}")
                      nx.append(t)
                  nw = wp.tile([128, D_IN], MMDT, tag="w1pf", name="w_l1_0")
                  nc.sync.dma_start(nw[:], w1[0:128, :D_IN])
                  for k in range(D_IN // 128):
                      x_engs[k % 3].dma_start(
                          nx[k][:], xt[k * 128 : (k + 1) * 128, :]
                      )
                  prefetched = (nx, nw)

              # --- final layer + scatter ---
              runs_by_tile = {}
              for r in runs:
                  runs_by_tile.setdefault(r[0], []).append(r)
              kt = H // 128
              # reverse order: the high-m tiles carry many small scatter runs;
              # doing them first lets those stores drain under later compute,
              # leaving a short 2-3-run tail on the last tile (m=0)
              for m in reversed(range(LT)):
                  wt = wp.tile([128, kt * 128], MMDT, tag="w", name=f"w_l5_{m}")
                  load_weight_block(wt, w3, kt, m, "l5")
                  if _N1024:
                      pst = ps.tile([128, BC], F32, tag="ps", name=f"ps_l5_{m}")
                      halves = [(pst[:, :NSZ], slice(0, NSZ)), (pst[:, NSZ:], slice(NSZ, BC))]
                      for k in range(kt):
                          nc.tensor.matmul(
                              pst[:],
                              wt[:, k * 128 : (k + 1) * 128],
                              h4[k][:, :],
                              start=(k == 0),
                              stop=(k == kt - 1),
                          )
                  else:
                      pst = ps.tile([128, BC], F32, tag="ps", name=f"ps_l5_{m}")
                      halves = [(pst[:], slice(0, BC))]
                      if _MM_ORDER == "halfmajor":
                          for csl in (slice(0, NSZ), slice(NSZ, BC)):
                              for k in range(kt):
                                  nc.tensor.matmul(
                                      pst[:, csl],
                                      wt[:, k * 128 : (k + 1) * 128],
                                      h4[k][:, csl],
                                      start=(k == 0),
                                      stop=(k == kt - 1),
                                  )
                      else:
                          for k in range(kt):
                              lhsT = wt[:, k * 128 : (k + 1) * 128]
                              nc.tensor.matmul(
                                  pst[:, :NSZ], lhsT, h4[k][:, :NSZ],
                                  start=(k == 0), stop=(k == kt - 1),
                              )
                              nc.tensor.matmul(
                                  pst[:, NSZ:], lhsT, h4[k][:, NSZ:],
                                  start=(k == 0), stop=(k == kt - 1),
                              )
                  y = yp.tile([128, BC], F32, tag="y", name=f"y_{m}")
                  for ps_ap, csl in halves:
                      nc.scalar.activation(
                          y[:, csl],
                          ps_ap,
                          ACT.Identity,
                          bias=bias_tiles["b3"][:, m : m + 1],
                      )
                  yc = ycp.tile([128, BC], F32, tag="yc", name=f"yc_{m}")
                  nc.vector.tensor_scalar_mul(yc[:], y[:], bias_tiles["c3"][:, m : m + 1])
                  nc.vector.tensor_max(y[:], y[:], yc[:])
                  # out stores: round-robin over the Pool (SWDGE) queue and the
                  # two HWDGE rings, which have slack during L5
                  out_engs = [nc.gpsimd, nc.sync, nc.scalar]
                  for ri, (_, p0, ln, r0) in enumerate(runs_by_tile.get(m, [])):
                      eng = out_engs[(m + ri) % 3]
                      eng.dma_start(out[r0 : r0 + ln, :], y[p0 : p0 + ln, :])

    nc.compile()
    return nc


def _get_nc():
    global _compiled_nc
    if _compiled_nc is None:
        _compiled_nc = _build()
    return _compiled_nc


def _tile_weight(W):
    """[K, M] -> [MT*128, KT*128] host pre-tiling (see _build docstring)."""
    K, M = W.shape
    kt, mt = K // 128, M // 128
    return np.ascontiguousarray(
        W.reshape(kt, 128, mt, 128).transpose(2, 1, 0, 3).reshape(mt * 128, kt * 128)
    )


def _np_mmdt():
    if _DTYPE == "bf16":
        import ml_dtypes

        return np.dtype(ml_dtypes.bfloat16)
    return np.dtype(np.float32)


def prepare_in_maps(x, W1, b1, W2, b2, W21, b21, W22, b22, W3, b3):
    mmdt = _np_mmdt()
    x = np.asarray(x, dtype=np.float32)
    perm, coef = _perm_and_coefs()
    w3p = np.zeros((H, LP), dtype=np.float32)
    w3p[:, :L] = np.asarray(W3, np.float32)[:, perm]
    b3p = np.zeros(LP, dtype=np.float32)
    b3p[:L] = np.asarray(b3, np.float32)[perm]

    def tile_bias(b, nt):
        return np.ascontiguousarray(np.asarray(b, np.float32).reshape(nt, 128).T)

    common = {
        "w1": _tile_weight(np.asarray(W1, np.float32).astype(mmdt)),
        "w2": _tile_weight(np.asarray(W2, np.float32).astype(mmdt)),
        "w21": _tile_weight(np.asarray(W21, np.float32).astype(mmdt)),
        "w22": _tile_weight(np.asarray(W22, np.float32).astype(mmdt)),
        "w3": _tile_weight(w3p.astype(mmdt)),
        "b1t": tile_bias(b1, 16),
        "b2t": tile_bias(b2, 16),
        "b21t": tile_bias(b21, 16),
        "b22t": tile_bias(b22, 16),
        "b3t": tile_bias(b3p, LT),
        "c3t": tile_bias(coef, LT),
    }
    xT = np.ascontiguousarray(x.T).astype(mmdt)  # [D_IN, B]
    return [
        {**common, "xt": np.ascontiguousarray(xT[:, c * BC : (c + 1) * BC])}
        for c in range(N_CORES)
    ]


def kernel(
    x, W1, b1, W2, b2, W21, b21, W22, b22, W3, b3
):  # noqa: N803 - match reference names
    nc = _get_nc()
    in_maps = prepare_in_maps(x, W1, b1, W2, b2, W21, b21, W22, b22, W3, b3)

    from concourse.bass_utils import run_bass_kernel_spmd

    res = run_bass_kernel_spmd(nc, in_maps, core_ids=list(range(N_CORES)))
    return np.concatenate(
        [res.results[c]["out"].reshape(CPV, CPV, BC) for c in range(N_CORES)], axis=2
    )



# revision 13
# speedup vs baseline: 1.0494x; 1.0494x over previous
"""Trainium2 Bass kernel for nn_ModelR_37022618091886.

Model: y = MLP(x) with 5 layers (leaky-relu 0.01 between), then per-example
triangular scatter of the 2080 outputs into an upper-triangular 64x64 matrix
(diagonal entries abs'ed), output shape (64, 64, 8192).

Strategy:
  - Data-parallel over batch across 8 cores (1024 examples/core), weights
    replicated.
  - Activations are kept feature-major on chip: h^T [features, batch], so the
    contraction dim of every matmul is already on SBUF partitions and the
    final layer directly produces y^T [2080, batch] = (almost) the output.
  - Matmuls run in bf16 (full PE rate + FWL weight loads; measured rel err
    ~5e-3 vs the f32 reference, well inside the 2e-2 gate).  PSUM/bias/output
    stay f32.  Weights are pre-tiled on the host to [MT*128, KT*128] so each
    m-tile's lhsT k-tiles load as one fully-contiguous [128, KT*128] DMA.
  - W3's columns are permuted on the host so that the final layer's output
    rows land in output-row order: row block i holds (i,j) for j=i..63
    ascending.  The "scatter" then degenerates to ~80 contiguous-run DMAs,
    spread round-robin over the Pool (SWDGE) queue and both HWDGE rings so
    the L5 weight stream is never blocked.  L5 m-tiles run in reverse order
    so the many-small-run tiles drain under later compute.
  - The diagonal abs is y = max(y, c*y) with c = -1 on diagonal rows else +1
    (per-partition scalar), fused after the bias add.
  - Strict-lower-triangle rows of the output are never written; the runtime
    pre-zeroes/donates zeroed output buffers (see run_bass_via_pjrt), so they
    read back as exact zeros.
  - PE is ~98% busy in the CoreSim timing model (499us of matmul stream per
    rep); the remaining ~12us are the x/w1 startup fill and the last
    m-tile's ACT->DVE->store tail.

HW findings (measured via microbenchmarks, 2026-08):
  - With all 8 cores running full PE streams the chip power-throttles the
    PE clock to ~2.0 GHz (per-MM N=512 ~250-265ns vs the 213.3ns 2.4 GHz
    ideal; a single busy core measures ~217ns/MM).  2336 matmuls x ~255ns
    = ~596us is the de-facto floor; the kernel sits on it.
  - LDWEIGHTS is fully hidden by the PE's pull-ahead even at one load per
    matmul (a kernel with a single LDW total measures the same per-MM).
    The legalizer still emits a redundant LDW for the second half-matmul
    of each (m,k) pair; _dedup_ldweights removes those 1168 (harmless now,
    helps if the clock ever runs unthrottled).
  - PSUM bank alternation per k (bank0/bank1 within a pair) has no HW
    penalty: sequential same-bank chains measure the same (seq2 variant).
  - fp8-e4m3 DoubleRow (1.44x PE rate) fails the 2e-2 gate: measured
    rel err 6-8e-2 end-to-end (even fp8 on only the middle layers), so
    bf16 dense is the fastest admissible algorithm.
  - Core-count sweep (6/7/8 busy cores) shows no per-MM difference, so
    8-way batch sharding (least work per core) is optimal; the clock
    state drifts ~20% on hour timescales regardless.
  - For reps > 1 (timing builds), the next rep's x tiles + w1-m0 are
    prefetched before the L5 store traffic so the next L1 doesn't stall
    behind ~8.5MB of output stores in the DMA ring FIFOs.  The w1
    prefetch uses its own pool tag ("w1pf"): sharing tag "w" would let
    L5's weight rotation claim the buffer and deadlock the PE queue.
"""

import os

import numpy as np

CPV = 64
L = CPV * (CPV + 1) // 2  # 2080
LT = 17  # number of 128-row tiles covering L (padded)
LP = LT * 128  # 2176
D_IN = 1024
H = 2048
B = 8192
N_CORES = 8
BC = B // N_CORES  # 1024 batch per core
NCH = 2  # moving-dim chunks of 512 (fp32 PSUM bank limit)
NSZ = BC // NCH

_DTYPE = os.environ.get("KERNEL_DTYPE", "bf16")  # "bf16" | "f32r" | "f32"
_REPS = int(os.environ.get("KERNEL_REPS", "1"))  # timing aid: unroll kernel R times
# matmul emission order within an m-tile:
#   kmajor   : per k, two N=512 matmuls alternating PSUM banks (original)
#   halfmajor: two sequential same-bank chains of kt matmuls (measured on HW:
#              no faster — there is no per-matmul PSUM bank-switch penalty)
_MM_ORDER = os.environ.get("KERNEL_MM_ORDER", "kmajor")
# N=1024 bf16 moving operands were tried (would halve the matmul/Ldweights
# instruction count) but neuronxcc rejects the 2-bank f32 PSUM output with an
# ISA check failure on TRN2 — the 128x1024 moving max evidently requires
# TRN3's 16-bit PSUM accumulation. Keep off.
_N1024 = _DTYPE == "bf16" and os.environ.get("KERNEL_N1024", "0") == "1"

_compiled_nc = None

# Remove the redundant second InstLdweights the legalizer emits for the
# b-half matmul of each (m,k) pair (identical weights AP, only matmuls in
# between on the PE queue).  Saves ~30ns of unhidden weight-load time per
# pair on HW.
_LDW_DEDUP = os.environ.get("KERNEL_LDW_DEDUP", "1") == "1"


def _dedup_ldweights(ordered):
    import concourse.mybir as mybir

    removed = 0
    rename = {}
    for bb, insts in ordered.items():
        keep = []
        last_sig = None
        last_name = None
        for inst in insts:
            if isinstance(inst, mybir.InstLdweights):
                sig = (
                    str(inst.ins[0]),
                    str(inst.perf_mode),
                    str(inst.is_transpose),
                    str(inst.tile_position),
                )
                if sig == last_sig and last_name is not None:
                    rename[inst.name] = last_name
                    removed += 1
                    continue
                last_sig = sig
                last_name = inst.name
            elif getattr(inst, "engine", None) == mybir.EngineType.PE:
                if not isinstance(inst, mybir.InstMatmult):
                    last_sig = None
                    last_name = None
            keep.append(inst)
        ordered[bb] = keep
    if rename:
        for bb, insts in ordered.items():
            for inst in insts:
                deps = set(inst.sync_dependency_names()) | set(
                    inst.nosync_dependency_names()
                )
                hit = {d: rename[d] for d in deps if d in rename}
                if hit:
                    inst.remap_dependency_names(hit)
    return ordered


def _install_dedup_patch(enable):
    import concourse.tile as tile_mod

    if not getattr(tile_mod, "_ldw_dedup_installed", False):
        orig = tile_mod.tile_legalize

        def patched(ordered, nc):
            out = orig(ordered, nc)
            if getattr(tile_mod, "_ldw_dedup_enabled", False):
                out = _dedup_ldweights(out)
            return out

        tile_mod.tile_legalize = patched
        tile_mod._ldw_dedup_installed = True
    tile_mod._ldw_dedup_enabled = enable


def _offsets():
    off = np.zeros(CPV + 1, dtype=np.int64)
    for i in range(CPV):
        off[i + 1] = off[i] + (CPV - i)
    return off


def _perm_and_coefs():
    """Column permutation for W3 + diag coefficient vector.

    New output order m: for i in 0..63, for j in i..63 -> m = off[i] + (j-i).
    Torch/ref order k: for i, for idx_y: col j = 63-idx_y -> k = off[i] + (63-j).
    """
    off = _offsets()
    perm = np.empty(L, dtype=np.int64)
    for i in range(CPV):
        n = CPV - i
        perm[off[i] : off[i] + n] = off[i] + np.arange(n)[::-1]
    coef = np.ones(LP, dtype=np.float32)
    coef[off[:CPV]] = -1.0  # diagonal (i,i) sits at the start of block i
    return perm, coef


def _out_runs():
    """Maximal runs of consecutive final-layer rows that map to consecutive
    output rows: list of (tile, p0, length, out_row0)."""
    off = _offsets()
    runs = []
    for t in range(LT):
        m0t, m1t = t * 128, min((t + 1) * 128, L)
        m = m0t
        while m < m1t:
            i = int(np.searchsorted(off, m, side="right") - 1)
            end = int(min(m1t, off[i] + (CPV - i)))
            runs.append((t, m - m0t, end - m, 65 * i + (m - off[i])))
            m = end
    assert sum(r[2] for r in runs) == L
    return runs


def _build(wp_bufs=6, ps_bufs=None, yp_bufs=3, reps=None, ldw_dedup=None):
    if ps_bufs is None:
        ps_bufs = 4  # [128,1024] f32 tiles span 2 banks each
    import concourse.bacc as bacc
    import concourse.mybir as mybir
    import concourse.tile as tile

    _install_dedup_patch(_LDW_DEDUP if ldw_dedup is None else ldw_dedup)

    F32 = mybir.dt.float32
    MMDT = {
        "bf16": mybir.dt.bfloat16,
        "f32r": mybir.dt.float32r,
        "f32": mybir.dt.float32,
    }[_DTYPE]
    ACT = mybir.ActivationFunctionType

    nc = bacc.Bacc("TRN2", target_bir_lowering=False, debug=False, num_devices=N_CORES)

    # weights are host-pre-tiled to [MT*128, KT*128]: row (m*128+p), col
    # (k*128+c) holds W[k*128+p, m*128+c], so loading one m-tile's lhsT
    # k-tiles is a single fully-contiguous [128, KT*128] DMA.
    xt = nc.dram_tensor("xt", [D_IN, BC], MMDT, kind="ExternalInput")
    w1 = nc.dram_tensor("w1", [H, D_IN], MMDT, kind="ExternalInput")
    w2 = nc.dram_tensor("w2", [H, H], MMDT, kind="ExternalInput")
    w21 = nc.dram_tensor("w21", [H, H], MMDT, kind="ExternalInput")
    w22 = nc.dram_tensor("w22", [H, H], MMDT, kind="ExternalInput")
    w3 = nc.dram_tensor("w3", [LP, H], MMDT, kind="ExternalInput")
    b1t = nc.dram_tensor("b1t", [128, H // 128], F32, kind="ExternalInput")
    b2t = nc.dram_tensor("b2t", [128, H // 128], F32, kind="ExternalInput")
    b21t = nc.dram_tensor("b21t", [128, H // 128], F32, kind="ExternalInput")
    b22t = nc.dram_tensor("b22t", [128, H // 128], F32, kind="ExternalInput")
    b3t = nc.dram_tensor("b3t", [128, LT], F32, kind="ExternalInput")
    c3t = nc.dram_tensor("c3t", [128, LT], F32, kind="ExternalInput")
    out = nc.dram_tensor("out", [CPV * CPV, BC], F32, kind="ExternalOutput")

    runs = _out_runs()

    with tile.TileContext(nc) as tc:
        with (
            tc.tile_pool(name="acts", bufs=1) as acts,
            tc.tile_pool(name="wp", bufs=wp_bufs) as wp,
            tc.tile_pool(name="cst", bufs=1) as cst,
            tc.tile_pool(name="yp", bufs=yp_bufs) as yp,
            tc.tile_pool(name="ycp", bufs=1) as ycp,
            tc.tile_pool(name="ps", bufs=ps_bufs, space="PSUM") as ps,
        ):
            # --- constants (biases / diag coefs): allocate now, load after
            # the startup-critical w1/x DMAs so they don't delay the first
            # Ldweights/matmul on any ring ---
            bias_specs = (
                ("b1", b1t, 16),
                ("b2", b2t, 16),
                ("b21", b21t, 16),
                ("b22", b22t, 16),
                ("b3", b3t, LT),
                ("c3", c3t, LT),
            )
            bias_tiles = {}
            for name, _dram, nt in bias_specs:
                t = cst.tile([128, nt], F32, tag=name, name=name)
                bias_tiles[name] = t

            n_reps = _REPS if reps is None else reps
            prefetched = None
            for _rep in range(n_reps):
              def load_weight_block(wt, w_dram, kt, m, lname):
                  """Load m-tile m's lhsT k-tiles: one contiguous [128, kt*128]
                  DMA, alternating between the two HWDGE rings per m-tile
                  (single DMA halves the completion-semaphore traffic)."""
                  eng = nc.sync if m % 2 == 0 else nc.scalar
                  eng.dma_start(wt[:], w_dram[m * 128 : (m + 1) * 128, : kt * 128])

              # startup: w1 m-tile 0 on the sync ring while x k0 loads in
              # parallel on the Pool ring; remaining x tiles (one DMA each)
              # round-robin over the three queues.  For reps > 1 these were
              # prefetched before the previous rep's L5 stores so they don't
              # queue behind them (w1-m0 gets its own pool tag "w1pf" so the
              # L5 weight rotation can't claim its buffer -> deadlock).
              x_engs = [nc.scalar, nc.gpsimd, nc.sync]
              if prefetched is None:
                  x_tiles = []
                  for k in range(D_IN // 128):
                    t = acts.tile([128, BC], MMDT, tag=f"x{k}", name=f"x{k}")
                    x_tiles.append(t)

                  w_l1_m0 = wp.tile([128, D_IN], MMDT, tag="w1pf", name="w_l1_0")
                  nc.sync.dma_start(w_l1_m0[:], w1[0:128, :D_IN])
                  nc.gpsimd.dma_start(x_tiles[0][:], xt[0:128, :])

                  for k in range(1, D_IN // 128):
                    x_engs[k % 3].dma_start(
                        x_tiles[k][:], xt[k * 128 : (k + 1) * 128, :]
                    )
              else:
                  x_tiles, w_l1_m0 = prefetched

              if _rep == 0:
                  # bias loads land behind the startup-critical DMAs, spread
                  # over the rings; b1 (first needed, at L1 m0's ACT ~6us in)
                  # leads on the sync ring
                  bias_engs = [nc.sync, nc.scalar, nc.gpsimd]
                  for bi, (name, dram, _nt) in enumerate(bias_specs):
                      bias_engs[bi % 3].dma_start(bias_tiles[name][:], dram[:, :])

              def layer(lname, w_dram, kt, mt, h_in, btile, out_tag_prefix, w_m0=None):
                  """One hidden layer: h_out[m] = lrelu(W[:,m]^T @ h_in + b[m])."""
                  h_out = []
                  for m in range(mt):
                      if m == 0 and w_m0 is not None:
                          wt = w_m0
                      else:
                          wt = wp.tile(
                              [128, kt * 128], MMDT, tag="w", name=f"w_{lname}_{m}"
                          )
                          load_weight_block(wt, w_dram, kt, m, lname)
                      if _N1024:
                          pst = ps.tile([128, BC], F32, tag="ps", name=f"ps_{lname}_{m}")
                          halves = [(pst[:, :NSZ], slice(0, NSZ)), (pst[:, NSZ:], slice(NSZ, BC))]
                          for k in range(kt):
                              nc.tensor.matmul(
                                  pst[:],
                                  wt[:, k * 128 : (k + 1) * 128],
                                  h_in[k][:, :],
                                  start=(k == 0),
                                  stop=(k == kt - 1),
                              )
                      else:
                          # one 2-bank PSUM tile per m-tile: each matmul's
                          # output stays within a single bank, but the two
                          # chains are adjacent so ONE activation reads both
                          pst = ps.tile([128, BC], F32, tag="ps", name=f"ps_{lname}_{m}")
                          halves = [(pst[:], slice(0, BC))]
                          if _MM_ORDER == "halfmajor":
                              # sequential same-bank chains (measured: same
                              # speed as kmajor — kept for reference only)
                              for csl in (slice(0, NSZ), slice(NSZ, BC)):
                                  for k in range(kt):
                                      nc.tensor.matmul(
                                          pst[:, csl],
                                          wt[:, k * 128 : (k + 1) * 128],
                                          h_in[k][:, csl],
                                          start=(k == 0),
                                          stop=(k == kt - 1),
                                      )
                          else:
                              for k in range(kt):
                                  lhsT = wt[:, k * 128 : (k + 1) * 128]
                                  nc.tensor.matmul(
                                      pst[:, :NSZ],
                                      lhsT,
                                      h_in[k][:, :NSZ],
                                      start=(k == 0),
                                      stop=(k == kt - 1),
                                  )
                                  nc.tensor.matmul(
                                      pst[:, NSZ:],
                                      lhsT,
                                      h_in[k][:, NSZ:],
                                      start=(k == 0),
                                      stop=(k == kt - 1),
                                  )
                      ht = acts.tile(
                          [128, BC],
                          MMDT,
                          tag=f"{out_tag_prefix}{m}",
                          name=f"h_{lname}_{m}",
                      )
                      for ps_ap, csl in halves:
                          nc.scalar.activation(
                              ht[:, csl],
                              ps_ap,
                              ACT.Lrelu,
                              bias=btile[:, m : m + 1],
                              scale=1.0,
                              alpha=0.01,
                          )
                      h_out.append(ht)
                  return h_out

              h1 = layer(
                  "l1", w1, D_IN // 128, H // 128, x_tiles, bias_tiles["b1"], "a",
                  w_m0=w_l1_m0,
              )
              h2 = layer("l2", w2, H // 128, H // 128, h1, bias_tiles["b2"], "b")
              h3 = layer("l3", w21, H // 128, H // 128, h2, bias_tiles["b21"], "a")
              h4 = layer("l4", w22, H // 128, H // 128, h3, bias_tiles["b22"], "b")

              # --- prefetch next rep's x + w1-m0 ahead of the L5 store
              # traffic (they would otherwise sit behind ~8.5MB of output
              # stores in the DMA ring FIFOs and stall the next rep's L1) ---
              prefetched = None
              if _rep + 1 < n_reps:
                  nx = []
                  for k in range(D_IN // 128):
                      t = acts.tile([128, BC], MMDT, tag=f"x{k}", name=f"x{k}")
                      nx.append(t)
                  nw = wp.tile([128, D_IN], MMDT, tag="w1pf", name="w_l1_0")
                  nc.sync.dma_start(nw[:], w1[0:128, :D_IN])
                  for k in range(D_IN // 128):
                      x_engs[k % 3].dma_start(
                          nx[k][:], xt[k * 128 : (k + 1) * 128, :]
                      )
                  prefetched = (nx, nw)

              # --- final layer + scatter ---
              runs_by_tile = {}
              for r in runs:
                  runs_by_tile.setdefault(r[0], []).append(r)
              kt = H // 128
              # reverse order: the high-m tiles carry many small scatter runs;
              # doing them first lets those stores drain under later compute,
              # leaving a short 2-3-run tail on the last tile (m=0)
              for m in reversed(range(LT)):
                  wt = wp.tile([128, kt * 128], MMDT, tag="w", name=f"w_l5_{m}")
                  load_weight_block(wt, w3, kt, m, "l5")
                  if _N1024:
                      pst = ps.tile([128, BC], F32, tag="ps", name=f"ps_l5_{m}")
                      halves = [(pst[:, :NSZ], slice(0, NSZ)), (pst[:, NSZ:], slice(NSZ, BC))]
                      for k in range(kt):
                          nc.tensor.matmul(
                              pst[:],
                              wt[:, k * 128 : (k + 1) * 128],
                              h4[k][:, :],
                              start=(k == 0),
                              stop=(k == kt - 1),
                          )
                  else:
                      pst = ps.tile([128, BC], F32, tag="ps", name=f"ps_l5_{m}")
                      halves = [(pst[:], slice(0, BC))]
                      if _MM_ORDER == "halfmajor":
                          for csl in (slice(0, NSZ), slice(NSZ, BC)):
                              for k in range(kt):
                                  nc.tensor.matmul(
                                      pst[:, csl],
                                      wt[:, k * 128 : (k + 1) * 128],
                                      h4[k][:, csl],
                                      start=(k == 0),
                                      stop=(k == kt - 1),
                                  )
                      else:
                          for k in range(kt):
                              lhsT = wt[:, k * 128 : (k + 1) * 128]
                              nc.tensor.matmul(
                                  pst[:, :NSZ], lhsT, h4[k][:, :NSZ],
                                  start=(k == 0), stop=(k == kt - 1),
                              )
                              nc.tensor.matmul(
                                  pst[:, NSZ:], lhsT, h4[k][:, NSZ:],
                                  start=(k == 0), stop=(k == kt - 1),
                              )
                  y = yp.tile([128, BC], F32, tag="y", name=f"y_{m}")
                  for ps_ap, csl in halves:
                      nc.scalar.activation(
                          y[:, csl],
                          ps_ap,
                          ACT.Identity,
                          bias=bias_tiles["b3"][:, m : m + 1],
                      )
                  yc = ycp.tile([128, BC], F32, tag="yc", name=f"yc_{m}")
                  nc.vector.tensor_scalar_mul(yc[:], y[:], bias_tiles["c3"][:, m : m + 1])
                  nc.vector.tensor_max(y[:], y[:], yc[:])
                  # out stores: round-robin over the Pool (SWDGE) queue and the
                  # two HWDGE rings, which have slack during L5
                  out_engs = [nc.gpsimd, nc.sync, nc.scalar]
                  for ri, (_, p0, ln, r0) in enumerate(runs_by_tile.get(m, [])):
                      eng = out_engs[(m + ri) % 3]
                      eng.dma_start(out[r0 : r0 + ln, :], y[p0 : p0 + ln, :])

    nc.compile()
    return nc


def _get_nc():
    global _compiled_nc
    if _compiled_nc is None:
        _compiled_nc = _build()
    return _compiled_nc


def _tile_weight(W):
    """[K, M] -> [MT*128, KT*128] host pre-tiling (see _build docstring)."""
    K, M = W.shape
    kt, mt = K // 128, M // 128
    return np.ascontiguousarray(
        W.reshape(kt, 128, mt, 128).transpose(2, 1, 0, 3).reshape(mt * 128, kt * 128)
    )


def _np_mmdt():
    if _DTYPE == "bf16":
        import ml_dtypes

        return np.dtype(ml_dtypes.bfloat16)
    return np.dtype(np.float32)


def prepare_in_maps(x, W1, b1, W2, b2, W21, b21, W22, b22, W3, b3):
    mmdt = _np_mmdt()
    x = np.asarray(x, dtype=np.float32)
    perm, coef = _perm_and_coefs()
    w3p = np.zeros((H, LP), dtype=np.float32)
    w3p[:, :L] = np.asarray(W3, np.float32)[:, perm]
    b3p = np.zeros(LP, dtype=np.float32)
    b3p[:L] = np.asarray(b3, np.float32)[perm]

    def tile_bias(b, nt):
        return np.ascontiguousarray(np.asarray(b, np.float32).reshape(nt, 128).T)

    common = {
        "w1": _tile_weight(np.asarray(W1, np.float32).astype(mmdt)),
        "w2": _tile_weight(np.asarray(W2, np.float32).astype(mmdt)),
        "w21": _tile_weight(np.asarray(W21, np.float32).astype(mmdt)),
        "w22": _tile_weight(np.asarray(W22, np.float32).astype(mmdt)),
        "w3": _tile_weight(w3p.astype(mmdt)),
        "b1t": tile_bias(b1, 16),
        "b2t": tile_bias(b2, 16),
        "b21t": tile_bias(b21, 16),
        "b22t": tile_bias(b22, 16),
        "b3t": tile_bias(b3p, LT),
        "c3t": tile_bias(coef, LT),
    }
    xT = np.ascontiguousarray(x.T).astype(mmdt)  # [D_IN, B]
    return [
        {**common, "xt": np.ascontiguousarray(xT[:, c * BC : (c + 1) * BC])}
        for c in range(N_CORES)
    ]


def kernel(
    x, W1, b1, W2, b2, W21, b21, W22, b22, W3, b3
):  # noqa: N803 - match reference names
    nc = _get_nc()
    in_maps = prepare_in_maps(x, W1, b1, W2, b2, W21, b21, W22, b22, W3, b3)

    from concourse.bass_utils import run_bass_kernel_spmd

    res = run_bass_kernel_spmd(nc, in_maps, core_ids=list(range(N_CORES)))
    return np.concatenate(
        [res.results[c]["out"].reshape(CPV, CPV, BC) for c in range(N_CORES)], axis=2
    )

